# revision 25
# baseline (speedup 1.0000x reference)
"""Trainium2 Bass kernel for the retrieval-KNN correlation problem.

Problem (per batch element b):
    idx[k,p]   = x[b,k,p] + 64*y[b,k,p]              (pixel coords into ref map)
    S[k,p]     = sum_c ref[b,c,idx[k,p]] * inp[b,c,p]
    best[p]    = argmax_k S[k,p]        (first occurrence on ties)
    out_x[p]   = x[b,best[p],p],  out_y[p] = y[b,best[p],p]

Sharding: 8 cores = (batch b = core//2, pixel half = core%2). Each core owns
all 16 candidates for 2048 contiguous pixels of one batch element, so there is
no cross-core communication.

Per-core dataflow (DMA-gather version):
  - ref[b] stays in DRAM, stored pixel-major (4096 rows x 256 channels, 1KB
    rows). The gather runs as SWDGE dma_gather: each int16 index fetches one
    contiguous 1KB row straight from HBM into SBUF (dst[i%128, i//128, :]).
    Two calls per candidate (1024 indices / 1MB each -- the SWDGE queue ring
    holds at most 1024 descriptors, HW-verified cliff); a call's descriptors
    spread across all 16 DMA engines, so each call runs at the ~360GB/s
    aggregate DMA roofline (~2.9us) and the 32 calls stream back-to-back
    (~93us total for 33.5MB). Descriptor generation on GPSIMD is cheap
    (994ns + 0.34ns/desc per call) and overlaps the previous transfer.
    This replaces the previous GPSIMD ap_gather ucode (~26ns/index serial on
    the Q7 cores, ~535us busy) -- the gather is now memory-bound.
  - Indices (x + 64*y as int16, wrapped in 16 partitions per the SWDGE index
    layout) and the pixel-major transposes of inp/x/y are precomputed on the
    host in make_in_maps, so the kernel has no on-chip index pipeline and no
    PE/PSUM use at all.
  - DVE consumes each gathered candidate as it lands: in-place multiply
    against the resident pixel-major inp tile, then a segmented 256->1
    add-reduce writes S directly in pixel-major order (stride-16 columns of
    st). ~3.7us per candidate, fully hidden under the next gather.
  - Final first-occurrence argmax via the reverse-weight trick + x/y select,
    all on DVE in pixel-major layout (no transposes needed).

HW-verified: exact match vs the jax reference (rel err 0.0).
"""

import numpy as np
from contextlib import ExitStack

import concourse.bacc as bacc
import concourse.bass as bass
import concourse.mybir as mybir
import concourse.tile as tile
from concourse import bass_utils

B, K, CN, H, W = 4, 16, 256, 64, 64
HW = H * W            # 4096 pixels per batch element
HALF = HW // 2        # 2048 pixels per core
NCORES = 8
NT = HALF // 128      # 16 pixel tiles of 128
NIW = HALF // 16      # 128 wrapped-index slots per candidate

f32 = mybir.dt.float32
i16 = mybir.dt.int16


def build_program():
    nc = bacc.Bacc("TRN2", target_bir_lowering=False, debug=False,
                   num_swdge_queues=4)

    refT_d = nc.dram_tensor("refT", (HW, CN), f32, kind="ExternalInput")
    it_d = nc.dram_tensor("it", (128, NT * CN), f32, kind="ExternalInput")
    wi_d = nc.dram_tensor("wi", (128, K * NIW), i16, kind="ExternalInput")
    xt_d = nc.dram_tensor("xt", (128, NT * K), f32, kind="ExternalInput")
    yt_d = nc.dram_tensor("yt", (128, NT * K), f32, kind="ExternalInput")
    revc_d = nc.dram_tensor("revc", (128, NT * K), f32, kind="ExternalInput")
    ox_d = nc.dram_tensor("ox", (128, NT), f32, kind="ExternalOutput")
    oy_d = nc.dram_tensor("oy", (128, NT), f32, kind="ExternalOutput")

    with ExitStack() as ctx:
        tc = ctx.enter_context(tile.TileContext(nc))
        pers = ctx.enter_context(tc.tile_pool(name="pers", bufs=1))
        gpool = ctx.enter_context(tc.tile_pool(name="g", bufs=6))

        # ---- persistent tiles -------------------------------------------------
        it = pers.tile([128, NT * CN], f32, tag="it")    # inp, pixel-major
        wi = pers.tile([128, K * NIW], i16, tag="wi")
        xt = pers.tile([128, NT * K], f32, tag="xt")     # x, pixel-major
        yt = pers.tile([128, NT * K], f32, tag="yt")
        revc = pers.tile([128, NT * K], f32, tag="revc")
        st = pers.tile([128, NT * K], f32, tag="st")     # S, pixel-major
        scr = pers.tile([128, CN], f32, tag="scr")       # scalar-engine scratch

        st_g = st[:].rearrange("p (t j) -> p t j", j=K)

        # All loads issue up front: they fill the DMA wire during the ~14us
        # GPSIMD ucode-library load that gates the first gather anyway.
        nc.sync.dma_start(wi[:], wi_d.ap())
        nc.sync.dma_start(it[:], it_d.ap())
        nc.sync.dma_start(xt[:], xt_d.ap())
        nc.sync.dma_start(yt[:], yt_d.ap())
        nc.sync.dma_start(revc[:], revc_d.ap())

        # 512-index chunks: the 1024-descriptor SWDGE ring then holds two
        # chunks per queue, so descriptor generation for the next chunk never
        # stalls on the previous chunk's drain; rotate across all 4 queues.
        CH = 512
        NCH = HALF // CH          # 4 chunks per candidate
        for k in range(K):
            g = gpool.tile([128, NT * CN], f32, tag="g", name=f"g{k}")
            for h2 in range(NCH):
                nc.gpsimd.dma_gather(
                    g[:, NT * CN // NCH * h2:NT * CN // NCH * (h2 + 1)]
                        .rearrange("p (j e) -> p j e", e=CN),
                    refT_d[:],
                    wi[:, k * NIW + (CH // 16) * h2:
                        k * NIW + (CH // 16) * (h2 + 1)],
                    CH, CH, CN,
                    queue_num=(k * NCH + h2) % 4,
                )
            # Whole-candidate multiply on DVE (the critical path). Reduces
            # alternate between DVE and the otherwise-idle Scalar engine,
            # whose activation accumulator sums one 256-channel segment per
            # call into st in fp32. (Finer-grained splits and other ratios
            # measured worse -- cross-engine sync overhead dominates.)
            nc.vector.tensor_mul(g[:], g[:], it[:])
            if k % 2 == 0:
                nc.vector.tensor_reduce(
                    st_g[:, :, k],
                    g[:].rearrange("p (j e) -> p j e", e=CN),
                    axis=mybir.AxisListType.X, op=mybir.AluOpType.add,
                )
            else:
                for j in range(NT):
                    nc.scalar.activation(
                        scr[:], g[:, j * CN:(j + 1) * CN],
                        mybir.ActivationFunctionType.Copy,
                        accum_out=st_g[:, j:j + 1, k],
                    )

        # ---- argmax (first occurrence) + offset select ------------------------
        def grp(ap):  # (128, 256) -> (128, 16, 16)
            return ap.rearrange("p (t j) -> p t j", j=K)

        gmax = pers.tile([128, NT], f32, tag="gmax")
        t1 = pers.tile([128, NT * K], f32, tag="t1")
        r1 = pers.tile([128, NT], f32, tag="r1")
        oh1 = pers.tile([128, NT * K], f32, tag="oh1")
        sel = pers.tile([128, NT * K], f32, tag="sel")
        oxv = pers.tile([128, NT], f32, tag="oxv")
        oyv = pers.tile([128, NT], f32, tag="oyv")

        nc.vector.tensor_reduce(gmax[:], st_g, axis=mybir.AxisListType.X,
                                op=mybir.AluOpType.max)
        gb = gmax[:].unsqueeze(2).broadcast_to((128, NT, K))
        nc.vector.tensor_tensor(grp(t1[:]), st_g, gb,
                                op=mybir.AluOpType.is_equal)
        nc.vector.tensor_mul(t1[:], t1[:], revc[:])
        nc.vector.tensor_reduce(r1[:], grp(t1[:]), axis=mybir.AxisListType.X,
                                op=mybir.AluOpType.max)
        rb = r1[:].unsqueeze(2).broadcast_to((128, NT, K))
        nc.vector.tensor_tensor(grp(oh1[:]), grp(t1[:]), rb,
                                op=mybir.AluOpType.is_equal)
        nc.vector.tensor_mul(sel[:], oh1[:], xt[:])
        nc.vector.tensor_reduce(oxv[:], grp(sel[:]), axis=mybir.AxisListType.X,
                                op=mybir.AluOpType.add)
        nc.vector.tensor_mul(sel[:], oh1[:], yt[:])
        nc.vector.tensor_reduce(oyv[:], grp(sel[:]), axis=mybir.AxisListType.X,
                                op=mybir.AluOpType.add)

        nc.sync.dma_start(ox_d.ap(), oxv[:])
        nc.sync.dma_start(oy_d.ap(), oyv[:])

    nc.compile()
    return nc


def make_consts():
    # revc[p, t*16 + j] = 16 - j : reverse weight for first-occurrence argmax
    return np.tile(
        np.tile((16.0 - np.arange(16, dtype=np.float32)), NT).reshape(1, NT * K),
        (128, 1),
    ).copy()


def make_in_maps(input_features, ref_features, aggregated_x, aggregated_y):
    revc = make_consts()
    in_maps = []
    for core in range(NCORES):
        b, h = core // 2, core % 2
        sl = slice(h * HALF, (h + 1) * HALF)
        inp = np.asarray(input_features[b]).reshape(CN, HW)[:, sl]
        xh = np.asarray(aggregated_x[b]).reshape(K, HW)[:, sl]
        yh = np.asarray(aggregated_y[b]).reshape(K, HW)[:, sl]
        idx = (xh + 64.0 * yh).astype(np.int16)              # (K, HALF)
        # SWDGE wrapped index layout: list elem i at partition i%16, slot
        # i//16, replicated across the 8 16-partition blocks.
        wi = np.tile(
            idx.reshape(K, NIW, 16).transpose(2, 0, 1).reshape(16, -1),
            (8, 1),
        )
        in_maps.append({
            "refT": np.ascontiguousarray(
                np.asarray(ref_features[b]).reshape(CN, HW).T),
            "it": np.ascontiguousarray(
                inp.reshape(CN, NT, 128).transpose(2, 1, 0).reshape(128, -1)),
            "wi": np.ascontiguousarray(wi),
            "xt": np.ascontiguousarray(
                xh.reshape(K, NT, 128).transpose(2, 1, 0).reshape(128, -1)),
            "yt": np.ascontiguousarray(
                yh.reshape(K, NT, 128).transpose(2, 1, 0).reshape(128, -1)),
            "revc": revc,
        })
    return in_maps


def assemble_outputs(results):
    offset_x = np.empty((B, 1, H, W), dtype=np.float32)
    offset_y = np.empty((B, 1, H, W), dtype=np.float32)
    for core in range(NCORES):
        b, h = core // 2, core % 2
        sl = slice(h * HALF, (h + 1) * HALF)
        # ox[p, t] holds pixel t*128+p -> transpose to pixel order
        offset_x[b, 0].reshape(HW)[sl] = results[core]["ox"].T.reshape(HALF)
        offset_y[b, 0].reshape(HW)[sl] = results[core]["oy"].T.reshape(HALF)
    return offset_x, offset_y


_PROGRAM = None


def kernel(input_features, ref_features, aggregated_x, aggregated_y):
    global _PROGRAM
    if _PROGRAM is None:
        _PROGRAM = build_program()
    nc = _PROGRAM
    in_maps = make_in_maps(input_features, ref_features, aggregated_x, aggregated_y)
    res = bass_utils.run_bass_kernel_spmd(nc, in_maps, core_ids=list(range(NCORES)))
    return assemble_outputs(res.results)


# revision 29
# speedup vs baseline: 1.2181x; 1.2181x over previous
"""Trainium2 Bass kernel for the retrieval-KNN correlation problem.

Problem (per batch element b):
    idx[k,p]   = x[b,k,p] + 64*y[b,k,p]              (pixel coords into ref map)
    S[k,p]     = sum_c ref[b,c,idx[k,p]] * inp[b,c,p]
    best[p]    = argmax_k S[k,p]        (first occurrence on ties)
    out_x[p]   = x[b,best[p],p],  out_y[p] = y[b,best[p],p]

Sharding: 8 cores = (batch b = core//2, pixel half = core%2). Each core owns
all 16 candidates for 2048 contiguous pixels of one batch element, so there is
no cross-core communication.

Per-core dataflow (DMA-gather version):
  - ref[b] stays in DRAM, stored pixel-major (4096 rows x 256 channels, 1KB
    rows). The gather runs as SWDGE dma_gather: each int16 index fetches one
    contiguous 1KB row straight from HBM into SBUF (dst[i%128, i//128, :]).
    Four calls per candidate (512 indices / 512KB each): the SWDGE queue
    ring holds at most 1024 descriptors (HW-verified cliff), so 512-index
    calls leave ring space for the next call's descriptors while the
    previous drains; calls rotate across the 4 SWDGE queues. A call's
    descriptors spread across all 16 DMA engines, so the stream runs at the
    ~0.3TB/s aggregate DMA roofline (~110us for 33.5MB). Descriptor
    generation on GPSIMD (~0.1-0.4us/call) overlaps previous transfers.
    This replaces the previous GPSIMD ap_gather ucode (~26ns/index serial on
    the Q7 cores, ~535us busy) -- the gather is now memory-bound.
  - Indices (x + 64*y as int16, wrapped in 16 partitions per the SWDGE index
    layout) and the pixel-major transposes of inp/x/y are precomputed on the
    host in make_in_maps, so the kernel has no on-chip index pipeline and no
    PE/PSUM use at all.
  - DVE consumes each gathered candidate as it lands: in-place multiply
    against the resident pixel-major inp tile (~4us; fp32 tensor_tensor is
    hard-limited to 1 elem/cycle/partition on DVE), then the 256->1
    add-reduce writes S directly in pixel-major order (stride-16 columns of
    st). Reduces alternate DVE (segmented tensor_reduce, ~3.6us) / Scalar
    engine (activation accum_out, 16 calls ~8us) so DVE+ACT together keep
    pace with the DMA wire (~110us/core for the 33.5MB gather).
  - Final first-occurrence argmax via the reverse-weight trick + x/y select,
    all on DVE in pixel-major layout (no transposes needed).

HW-verified: exact match vs the jax reference (rel err 0.0), ~154-165us
(run-to-run HW variance; was 622us with the GPSIMD ap_gather design).
"""

import numpy as np
from contextlib import ExitStack

import concourse.bacc as bacc
import concourse.bass as bass
import concourse.mybir as mybir
import concourse.tile as tile
from concourse import bass_utils

B, K, CN, H, W = 4, 16, 256, 64, 64
HW = H * W            # 4096 pixels per batch element
HALF = HW // 2        # 2048 pixels per core
NCORES = 8
NT = HALF // 128      # 16 pixel tiles of 128
NIW = HALF // 16      # 128 wrapped-index slots per candidate

f32 = mybir.dt.float32
i16 = mybir.dt.int16


def build_program():
    nc = bacc.Bacc("TRN2", target_bir_lowering=False, debug=False,
                   num_swdge_queues=4)

    refT_d = nc.dram_tensor("refT", (HW, CN), f32, kind="ExternalInput")
    it_d = nc.dram_tensor("it", (128, NT * CN), f32, kind="ExternalInput")
    wi_d = nc.dram_tensor("wi", (128, K * NIW), i16, kind="ExternalInput")
    xt_d = nc.dram_tensor("xt", (128, NT * K), f32, kind="ExternalInput")
    yt_d = nc.dram_tensor("yt", (128, NT * K), f32, kind="ExternalInput")
    revc_d = nc.dram_tensor("revc", (128, NT * K), f32, kind="ExternalInput")
    ox_d = nc.dram_tensor("ox", (128, NT), f32, kind="ExternalOutput")
    oy_d = nc.dram_tensor("oy", (128, NT), f32, kind="ExternalOutput")

    with ExitStack() as ctx:
        tc = ctx.enter_context(tile.TileContext(nc))
        pers = ctx.enter_context(tc.tile_pool(name="pers", bufs=1))
        gpool = ctx.enter_context(tc.tile_pool(name="g", bufs=6))

        # ---- persistent tiles -------------------------------------------------
        it = pers.tile([128, NT * CN], f32, tag="it")    # inp, pixel-major
        wi = pers.tile([128, K * NIW], i16, tag="wi")
        xt = pers.tile([128, NT * K], f32, tag="xt")     # x, pixel-major
        yt = pers.tile([128, NT * K], f32, tag="yt")
        revc = pers.tile([128, NT * K], f32, tag="revc")
        st = pers.tile([128, NT * K], f32, tag="st")     # S, pixel-major
        scr = pers.tile([128, CN], f32, tag="scr")       # scalar-engine scratch

        st_g = st[:].rearrange("p (t j) -> p t j", j=K)

        # All loads issue up front: they fill the DMA wire during the ~14us
        # GPSIMD ucode-library load that gates the first gather anyway.
        nc.sync.dma_start(wi[:], wi_d.ap())
        nc.sync.dma_start(it[:], it_d.ap())
        nc.sync.dma_start(xt[:], xt_d.ap())
        nc.sync.dma_start(yt[:], yt_d.ap())
        nc.sync.dma_start(revc[:], revc_d.ap())

        # 512-index chunks: the 1024-descriptor SWDGE ring then holds two
        # chunks per queue, so descriptor generation for the next chunk never
        # stalls on the previous chunk's drain; rotate across all 4 queues.
        CH = 512
        NCH = HALF // CH          # 4 chunks per candidate
        for k in range(K):
            g = gpool.tile([128, NT * CN], f32, tag="g", name=f"g{k}")
            for h2 in range(NCH):
                nc.gpsimd.dma_gather(
                    g[:, NT * CN // NCH * h2:NT * CN // NCH * (h2 + 1)]
                        .rearrange("p (j e) -> p j e", e=CN),
                    refT_d[:],
                    wi[:, k * NIW + (CH // 16) * h2:
                        k * NIW + (CH // 16) * (h2 + 1)],
                    CH, CH, CN,
                    queue_num=(k * NCH + h2) % 4,
                )
            # Whole-candidate multiply on DVE (the critical path). Reduces
            # alternate between DVE and the otherwise-idle Scalar engine,
            # whose activation accumulator sums one 256-channel segment per
            # call into st in fp32. (Finer-grained splits and other ratios
            # measured worse -- cross-engine sync overhead dominates.) The
            # last candidate runs as two half-sized DVE ops so its consume
            # starts as soon as half its chunks land, shortening the tail.
            if k == K - 1:
                for h2 in range(2):
                    gh = g[:, NT * CN // 2 * h2:NT * CN // 2 * (h2 + 1)]
                    ih = it[:, NT * CN // 2 * h2:NT * CN // 2 * (h2 + 1)]
                    nc.vector.tensor_mul(gh, gh, ih)
                    nc.vector.tensor_reduce(
                        st_g[:, NT // 2 * h2:NT // 2 * (h2 + 1), k],
                        gh.rearrange("p (j e) -> p j e", e=CN),
                        axis=mybir.AxisListType.X, op=mybir.AluOpType.add,
                    )
            else:
                nc.vector.tensor_mul(g[:], g[:], it[:])
                if k % 2 == 0:
                    nc.vector.tensor_reduce(
                        st_g[:, :, k],
                        g[:].rearrange("p (j e) -> p j e", e=CN),
                        axis=mybir.AxisListType.X, op=mybir.AluOpType.add,
                    )
                else:
                    for j in range(NT):
                        nc.scalar.activation(
                            scr[:], g[:, j * CN:(j + 1) * CN],
                            mybir.ActivationFunctionType.Copy,
                            accum_out=st_g[:, j:j + 1, k],
                        )

        # ---- argmax (first occurrence) + offset select ------------------------
        def grp(ap):  # (128, 256) -> (128, 16, 16)
            return ap.rearrange("p (t j) -> p t j", j=K)

        gmax = pers.tile([128, NT], f32, tag="gmax")
        t1 = pers.tile([128, NT * K], f32, tag="t1")
        r1 = pers.tile([128, NT], f32, tag="r1")
        oh1 = pers.tile([128, NT * K], f32, tag="oh1")
        sel = pers.tile([128, NT * K], f32, tag="sel")
        oxv = pers.tile([128, NT], f32, tag="oxv")
        oyv = pers.tile([128, NT], f32, tag="oyv")

        nc.vector.tensor_reduce(gmax[:], st_g, axis=mybir.AxisListType.X,
                                op=mybir.AluOpType.max)
        gb = gmax[:].unsqueeze(2).broadcast_to((128, NT, K))
        nc.vector.tensor_tensor(grp(t1[:]), st_g, gb,
                                op=mybir.AluOpType.is_equal)
        nc.vector.tensor_mul(t1[:], t1[:], revc[:])
        nc.vector.tensor_reduce(r1[:], grp(t1[:]), axis=mybir.AxisListType.X,
                                op=mybir.AluOpType.max)
        rb = r1[:].unsqueeze(2).broadcast_to((128, NT, K))
        nc.vector.tensor_tensor(grp(oh1[:]), grp(t1[:]), rb,
                                op=mybir.AluOpType.is_equal)
        nc.vector.tensor_mul(sel[:], oh1[:], xt[:])
        nc.vector.tensor_reduce(oxv[:], grp(sel[:]), axis=mybir.AxisListType.X,
                                op=mybir.AluOpType.add)
        nc.vector.tensor_mul(sel[:], oh1[:], yt[:])
        nc.vector.tensor_reduce(oyv[:], grp(sel[:]), axis=mybir.AxisListType.X,
                                op=mybir.AluOpType.add)

        nc.sync.dma_start(ox_d.ap(), oxv[:])
        nc.sync.dma_start(oy_d.ap(), oyv[:])

    nc.compile()
    return nc


def make_consts():
    # revc[p, t*16 + j] = 16 - j : reverse weight for first-occurrence argmax
    return np.tile(
        np.tile((16.0 - np.arange(16, dtype=np.float32)), NT).reshape(1, NT * K),
        (128, 1),
    ).copy()


def make_in_maps(input_features, ref_features, aggregated_x, aggregated_y):
    revc = make_consts()
    in_maps = []
    for core in range(NCORES):
        b, h = core // 2, core % 2
        sl = slice(h * HALF, (h + 1) * HALF)
        inp = np.asarray(input_features[b]).reshape(CN, HW)[:, sl]
        xh = np.asarray(aggregated_x[b]).reshape(K, HW)[:, sl]
        yh = np.asarray(aggregated_y[b]).reshape(K, HW)[:, sl]
        idx = (xh + 64.0 * yh).astype(np.int16)              # (K, HALF)
        # SWDGE wrapped index layout: list elem i at partition i%16, slot
        # i//16, replicated across the 8 16-partition blocks.
        wi = np.tile(
            idx.reshape(K, NIW, 16).transpose(2, 0, 1).reshape(16, -1),
            (8, 1),
        )
        in_maps.append({
            "refT": np.ascontiguousarray(
                np.asarray(ref_features[b]).reshape(CN, HW).T),
            "it": np.ascontiguousarray(
                inp.reshape(CN, NT, 128).transpose(2, 1, 0).reshape(128, -1)),
            "wi": np.ascontiguousarray(wi),
            "xt": np.ascontiguousarray(
                xh.reshape(K, NT, 128).transpose(2, 1, 0).reshape(128, -1)),
            "yt": np.ascontiguousarray(
                yh.reshape(K, NT, 128).transpose(2, 1, 0).reshape(128, -1)),
            "revc": revc,
        })
    return in_maps


def assemble_outputs(results):
    offset_x = np.empty((B, 1, H, W), dtype=np.float32)
    offset_y = np.empty((B, 1, H, W), dtype=np.float32)
    for core in range(NCORES):
        b, h = core // 2, core % 2
        sl = slice(h * HALF, (h + 1) * HALF)
        # ox[p, t] holds pixel t*128+p -> transpose to pixel order
        offset_x[b, 0].reshape(HW)[sl] = results[core]["ox"].T.reshape(HALF)
        offset_y[b, 0].reshape(HW)[sl] = results[core]["oy"].T.reshape(HALF)
    return offset_x, offset_y


_PROGRAM = None


def kernel(input_features, ref_features, aggregated_x, aggregated_y):
    global _PROGRAM
    if _PROGRAM is None:
        _PROGRAM = build_program()
    nc = _PROGRAM
    in_maps = make_in_maps(input_features, ref_features, aggregated_x, aggregated_y)
    res = bass_utils.run_bass_kernel_spmd(nc, in_maps, core_ids=list(range(NCORES)))
    return assemble_outputs(res.results)
